# revision 41
# baseline (speedup 1.0000x reference)
"""Trainium2 Bass kernel for nn_ArgumentClassification (2-layer BiLSTM tagger).

Sharding: 8 cores = 4 batch slices x 2 directions. Core c handles batch rows
[c//2*8 : c//2*8+8] and direction ('f' if c%2==0 else 'b'). Backward cores
receive their inputs TIME-REVERSED on the host, so the device program is
identical on every core (pure forward scan); the host un-reverses and sums
the per-direction partial outputs.

This halves the per-core scan weight-load volume vs. batch-only sharding:
the 256-step LSTM recurrence is bound by streaming Whh (2048x512 bf16)
through the PE array every step (~45ns per ldweights+matmul pair), so one
direction per core = 64 pairs/step instead of 128.

Mid-kernel exchange: layer 1 consumes [h0f; h0b]. Each core stores a
time-reversed copy of its layer-0 output (hdst2) -- reversed-in-my-frame is
exactly the peer's time convention -- and the fwd/bwd core pairs AllReduce
their hdst2 through HBM; subtracting one's own contribution leaves the
peer's h0, time-aligned locally. The output projection splits by k:
out = h1f @ Wf.T + (h1b @ Wb.T reversed), summed on the host, so no second
exchange is needed.

Per-core pipeline:
  1. x.T features [128, 7, SB] built from the mean of 4 transformer layers
     (PE transposes), plus delta/mask/ones feature rows. The predicate
     one-hot and role mask are host-precomputed (tiny [B,S] int ops).
  2. L0 input projection (batched matmuls, bias folded via ones-row).
  3. L0 scan: 256 steps in gates-transposed layout [2048, BL], Whh
     stationary, gate groups in (g,i,f,o) order so the DVE/ACT nonlinearity
     chain of each group overlaps the next group's matmuls.
  4. hdst2 AllReduce with pair core; hrecv = sum - hdst2.
  5. L1 projection from [hdst(own); hrecv(peer)] + bias, L1 scan.
  6. out partial = h1 @ W_out[own half].T (+ bias on fwd cores only),
     PE-transposed to [BL, S, 30] and DMA'd out.

Gate order is host-permuted from PyTorch's (i,f,g,o) to (i,f,o,g).
"""
import sys

sys.path.insert(0, "/opt/trn_rl_repo")

import numpy as np
import ml_dtypes

import concourse.bass as bass
import concourse.tile as tile
from concourse import bacc, mybir
from concourse.bass import ds
from concourse.masks import make_identity

BF16 = mybir.dt.bfloat16
F32 = mybir.dt.float32
AF = mybir.ActivationFunctionType
OP = mybir.AluOpType

B, S, E, H, C = 32, 256, 768, 512, 30
NCORES = 8
NPAIR = 4                 # batch slices (pairs of cores)
BL = B // NPAIR           # 8 rows per core
SB = S * BL               # 2048 columns, ordered (t, b): col = t*BL + b
G = 4 * H                 # 2048 gate rows
MT = G // 128             # 16 gate m-tiles
KH = H // 128             # 4 hidden k-tiles
K0 = 7                    # L0 input k-tiles ([770 + ones-row] padded to 896)
K1O = 5                   # L1 own-half k-tiles (512 + bias row -> 640)
K1R = 4                   # L1 recv-half k-tiles (512)
RG = [[0, 1], [2, 3], [4, 5], [6, 7]]

_cache = {}


def _bf(a):
    return np.asarray(a, dtype=ml_dtypes.bfloat16)


def _prep_weights(inp, d):
    """Host-side weight prep for direction d ('f'/'b'): permute gates to
    (i,f,o,g), transpose, pad, fold biases, tile for SBUF."""
    perm = np.concatenate([
        np.arange(0, H),          # i
        np.arange(H, 2 * H),      # f
        np.arange(3 * H, 4 * H),  # o
        np.arange(2 * H, 3 * H),  # g
    ])
    out = {}

    def tile_k(a, nk):
        # [nk*128, M] -> [128, nk, M]
        return np.ascontiguousarray(
            a.reshape(nk, 128, a.shape[1]).transpose(1, 0, 2))

    def tile_km(a, nk):
        # [nk*128, 16*128] -> [16, 128, nk, 128]  (per-m-block contiguous)
        m = a.shape[1] // 128
        return np.ascontiguousarray(
            a.reshape(nk, 128, m, 128).transpose(2, 1, 0, 3))

    own = slice(0, H) if d == "f" else slice(H, 2 * H)
    rcv = slice(H, 2 * H) if d == "f" else slice(0, H)

    # layer 0
    wih = inp[f"Wih_l0{d}"][perm]                     # [2048, 770]
    bias = (inp[f"bih_l0{d}"] + inp[f"bhh_l0{d}"])[perm]
    ext = np.zeros((K0 * 128, G), np.float32)
    ext[:768] = wih.T[:768]
    ext[768] = wih.T[768]      # delta coeffs at tile6 partition 0
    ext[800] = wih.T[769]      # mask coeffs at tile6 partition 32
    ext[832] = bias            # bias row at tile6 partition 64
    out["wih0"] = _bf(tile_km(ext, K0))               # [16,128,7,128]
    whh = inp[f"Whh_l0{d}"][perm]                     # [2048, 512]
    out["whh0"] = _bf(tile_k(whh.T, KH))              # [128, 4, 2048]

    # layer 1, split into own-half (+bias) and recv-half
    w1T = inp[f"Wih_l1{d}"][perm].T                   # [1024, 2048]
    bias = (inp[f"bih_l1{d}"] + inp[f"bhh_l1{d}"])[perm]
    ext = np.zeros((K1O * 128, G), np.float32)
    ext[:512] = w1T[own]
    ext[512] = bias
    out["wih1o"] = _bf(tile_km(ext, K1O))             # [16,128,5,128]
    ext = np.zeros((K1R * 128, G), np.float32)
    ext[:512] = w1T[rcv]
    out["wih1r"] = _bf(tile_km(ext, K1R))             # [16,128,4,128]
    whh = inp[f"Whh_l1{d}"][perm]
    out["whh1"] = _bf(tile_k(whh.T, KH))

    # output projection own half [640, 30]; bias only on fwd cores
    ext = np.zeros((K1O * 128, C), np.float32)
    ext[:512] = inp["W_out"].T[own]
    if d == "f":
        ext[512] = inp["b_out"]
    out["wout"] = _bf(tile_k(ext, K1O))               # [128, 5, 30]
    return out


def _prep_core_inputs(inputs, wmaps, core):
    pair, parity = core // 2, core % 2
    d = "f" if parity == 0 else "b"
    rows = slice(BL * pair, BL * (pair + 1))

    hs = np.asarray(inputs["hidden_states"], np.float32)[:, rows]  # [4,BL,S,E]
    roles = np.asarray(inputs["roles"])[rows]                      # [BL,S]
    preds = np.asarray(inputs["predicates"])[rows]
    rmask = ((roles != 0) & (roles != -100)).astype(np.float32)
    idx = np.argmax(preds, axis=-1)                                # [BL]
    mw = hs.mean(axis=0).mean(axis=-1)                             # [BL,S]
    delta = (mw - np.take_along_axis(mw, idx[:, None], 1)).astype(np.float32)
    if parity == 1:  # time-reverse for backward cores
        hs = hs[:, :, ::-1]
        rmask = rmask[:, ::-1]
        delta = delta[:, ::-1]
    m = dict(wmaps[d])
    m["hs"] = _bf(hs)
    m["rmask"] = np.ascontiguousarray(rmask.T).reshape(1, SB)      # (t,b)
    m["drow"] = np.ascontiguousarray(delta.T).reshape(1, SB)
    return m


def build_nc():
    nc = bacc.Bacc("TRN2", target_bir_lowering=False, debug=False,
                   num_devices=NCORES)
    hs = nc.dram_tensor("hs", [4, BL, S, E], BF16, kind="ExternalInput").ap()
    rmask = nc.dram_tensor("rmask", [1, SB], F32, kind="ExternalInput").ap()
    drow = nc.dram_tensor("drow", [1, SB], F32, kind="ExternalInput").ap()
    w = {}
    w["wih0"] = nc.dram_tensor("wih0", [MT, 128, K0, 128], BF16,
                               kind="ExternalInput").ap()
    w["wih1o"] = nc.dram_tensor("wih1o", [MT, 128, K1O, 128], BF16,
                                kind="ExternalInput").ap()
    w["wih1r"] = nc.dram_tensor("wih1r", [MT, 128, K1R, 128], BF16,
                                kind="ExternalInput").ap()
    w["whh0"] = nc.dram_tensor("whh0", [128, KH, G], BF16,
                               kind="ExternalInput").ap()
    w["whh1"] = nc.dram_tensor("whh1", [128, KH, G], BF16,
                               kind="ExternalInput").ap()
    w["wout"] = nc.dram_tensor("wout", [128, K1O, C], BF16,
                               kind="ExternalInput").ap()
    hx = {}
    for half in ("hi", "lo"):
        hx[f"in_{half}"] = nc.dram_tensor(f"hx_in_{half}", [128, KH, SB // 2],
                                          BF16, kind="Internal")
        hx[f"out_{half}"] = nc.dram_tensor(f"hx_out_{half}",
                                           [128, KH, SB // 2], BF16,
                                           kind="Internal")
    out = nc.dram_tensor("out", [BL, S, C], F32, kind="ExternalOutput").ap()

    with tile.TileContext(nc) as tc:
        _emit(nc, tc, hs, rmask, drow, w, hx, out)
    nc.compile()
    return nc


class _Filler:
    """Deadline-aware FIFO of emission thunks. Items are emitted between
    scan steps so their PE work lands in the scan's dependency-stall gaps.
    Strict FIFO pops keep PE program order consistent with producer ->
    consumer order (no in-order-engine deadlocks)."""

    def __init__(self):
        self.q = []

    def add(self, earliest, deadline, fn):
        self.q.append((earliest, deadline, fn))

    def step(self, j, budget=1):
        n = 0
        while self.q and self.q[0][1] <= j:
            self.q.pop(0)[2]()
            n += 1
        while self.q and n < budget and self.q[0][0] <= j:
            self.q.pop(0)[2]()
            n += 1

    def drain(self):
        while self.q:
            self.q.pop(0)[2]()


def _emit(nc, tc, hs, rmask, drow, w, hx, out):
    from contextlib import ExitStack
    NCH = SB // 512          # 4 column chunks (64 timesteps each)
    SPC = S // NCH           # 64 scan steps per chunk
    NRT = SB // 128          # 16 (t,b) row-tiles
    RPC = NRT // NCH         # 4 row-tiles per chunk
    TPR = 128 // BL          # 16 timesteps per row-tile
    with ExitStack() as st:
        cpool = st.enter_context(tc.tile_pool(name="const", bufs=1))
        rpool = st.enter_context(tc.tile_pool(name="rows", bufs=1))
        xwpool = st.enter_context(tc.tile_pool(name="xw", bufs=1))
        scpool = st.enter_context(tc.tile_pool(name="sc", bufs=3))
        wpool = st.enter_context(tc.tile_pool(name="wts", bufs=2))
        pspool = st.enter_context(tc.tile_pool(name="ps", bufs=1, space="PSUM"))
        psproj = st.enter_context(tc.tile_pool(name="psp", bufs=2,
                                               space="PSUM"))
        psg = st.enter_context(tc.tile_pool(name="psg", bufs=5, space="PSUM"))

        ident = cpool.tile([128, 128], F32, tag="ident")
        make_identity(nc, ident[:, :])
        ones_row = cpool.tile([128, SB], BF16, tag="onesrow")
        nc.vector.memset(ones_row[:, :], 0.0)
        nc.vector.memset(ones_row[0:1, :], 1.0)

        whh0 = wpool.tile([128, KH, G], BF16, tag="whh", name="whh0")
        nc.sync.dma_start(out=whh0[:, :, :], in_=w["whh0"][:, :, :])
        hdst = rpool.tile([128, KH, SB], BF16, tag="hdst", name="hdst")
        hdst2 = rpool.tile([128, KH, SB], BF16, tag="hdst2", name="hdst2")

        filler = _Filler()

        def scan_layer(whh_sb, xw, hd, hd2):
            """Single-direction 256-step scan with gap-filler items."""
            hbuf = rpool.tile([128, 2, KH, BL], BF16, tag="hbuf", name="hbuf")
            nc.vector.memset(hbuf[:, 0, :, :], 0.0)
            cbuf = rpool.tile([128, KH, BL], F32, tag="cbuf", name="cbuf")
            nc.vector.memset(cbuf[:, :, :], 0.0)

            for j in range(S):
                filler.step(j)
                cur, nxt = j % 2, (j + 1) % 2
                cols = j * BL
                gorder = (3, 0, 1, 2)          # g, i, f, o
                pg, gs = {}, {}
                for gg in gorder:
                    pg[gg] = psg.tile([128, KH, BL], F32, tag="gates",
                                      name=f"pg{gg}")
                    for mm in range(KH):
                        m = 4 * gg + mm
                        for k in range(KH):
                            nc.tensor.matmul(
                                pg[gg][:, mm, :],
                                whh_sb[:, k, 128 * m:128 * (m + 1)],
                                hbuf[:, cur, k, :],
                                start=(k == 0), stop=(k == KH - 1))
                    gs[gg] = scpool.tile([128, KH, BL], F32, tag=f"gs{gg}",
                                         name=f"gs{gg}")
                    nc.vector.tensor_tensor(
                        gs[gg][:, :, :], pg[gg][:, :, :],
                        xw[:, 4 * gg:4 * gg + 4, ds(cols, BL)], OP.add)
                    if gg == 3:
                        tg = scpool.tile([128, KH, BL], F32, tag="tg",
                                         name="tg")
                        nc.scalar.activation(tg[:, :, :], gs[3][:, :, :],
                                             AF.Tanh)
                    elif gg == 0:
                        si = scpool.tile([128, KH, BL], F32, tag="si")
                        nc.scalar.activation(si[:, :, :], gs[0][:, :, :],
                                             AF.Sigmoid)
                    elif gg == 1:
                        # t1 emitted here (not in the i-branch) so gs_f is
                        # not queued behind it on the DVE: gs_f can issue
                        # the moment the f matmuls complete.
                        sf = scpool.tile([128, KH, BL], F32, tag="sf")
                        nc.scalar.activation(sf[:, :, :], gs[1][:, :, :],
                                             AF.Sigmoid)
                        t1 = scpool.tile([128, KH, BL], F32, tag="t1",
                                         name="t1")
                        nc.vector.tensor_tensor(t1[:, :, :], si[:, :, :],
                                                tg[:, :, :], OP.mult)
                        t2 = scpool.tile([128, KH, BL], F32, tag="t2")
                        nc.vector.tensor_tensor(t2[:, :, :], sf[:, :, :],
                                                cbuf[:, :, :], OP.mult)
                        nc.vector.tensor_tensor(cbuf[:, :, :], t1[:, :, :],
                                                t2[:, :, :], OP.add)
                        tcc = scpool.tile([128, KH, BL], F32, tag="tcc",
                                          name="tcc")
                        nc.scalar.activation(tcc[:, :, :], cbuf[:, :, :],
                                             AF.Tanh)
                    else:
                        so = scpool.tile([128, KH, BL], F32, tag="so",
                                         name="so")
                        nc.scalar.activation(so[:, :, :], gs[2][:, :, :],
                                             AF.Sigmoid)
                        nc.vector.tensor_tensor(hbuf[:, nxt, :, :],
                                                so[:, :, :], tcc[:, :, :],
                                                OP.mult)
                nc.vector.tensor_tensor(hd[:, :, ds(cols, BL)],
                                        so[:, :, :], tcc[:, :, :], OP.mult)
                if hd2 is not None:
                    nc.vector.tensor_tensor(
                        hd2[:, :, ds((S - 1 - j) * BL, BL)],
                        so[:, :, :], tcc[:, :, :], OP.mult)

        # ---- layer 0: x.T features + projection, chunk-pipelined ---------
        xw0 = xwpool.tile([128, MT, SB], BF16, tag="xw", name="xw0")
        stx = st.enter_context(ExitStack())
        xtpool = stx.enter_context(tc.tile_pool(name="xtp", bufs=1))
        hlpool = stx.enter_context(tc.tile_pool(name="hl", bufs=5))
        sumpool = stx.enter_context(tc.tile_pool(name="sum", bufs=4))
        frpool = stx.enter_context(tc.tile_pool(name="frp", bufs=1))

        xt = xtpool.tile([128, K0, SB], BF16, tag="xt")
        hs_sbe = hs.rearrange("l b s e -> l s b e")
        rt_sum = {}

        def rowtile_dma(r):
            hl = []
            for layer in range(4):
                t = hlpool.tile([128, E], BF16, tag="hl")
                nc.sync.dma_start(
                    out=t[:, :],
                    in_=hs_sbe[layer, TPR * r:TPR * (r + 1), :, :])
                hl.append(t)
            s01 = sumpool.tile([128, E], F32, tag="sum")
            nc.vector.tensor_tensor(s01[:, :], hl[0][:, :], hl[1][:, :],
                                    OP.add)
            s23 = sumpool.tile([128, E], F32, tag="sum")
            nc.vector.tensor_tensor(s23[:, :], hl[2][:, :], hl[3][:, :],
                                    OP.add)
            ssum = sumpool.tile([128, E], F32, tag="sum")
            nc.vector.tensor_tensor(ssum[:, :], s01[:, :], s23[:, :], OP.add)
            rt_sum[r] = ssum

        def rowtile_tp(r):
            ssum = rt_sum.pop(r)
            for c in range(6):
                pt = pspool.tile([128, 128], F32, tag="tp")
                nc.tensor.transpose(pt[:, :], ssum[:, 128 * c:128 * (c + 1)],
                                    ident[:, :])
                nc.vector.tensor_scalar_mul(
                    xt[:, c, 128 * r:128 * (r + 1)], pt[:, :], 0.25)

        def proj_item(xw, ch, segs, off=0, ln=512):
            """One m-tile, one column sub-range of a 512-col chunk: weight
            DMAs + one accumulation chain + PSUM->SBUF copy. segs: list of
            (wih_dram, nk, rhs_of_k, m). Returns a thunk."""
            def emit():
                wms = []
                for (wih_dram, nk, _, m) in segs:
                    wm = wpool.tile([128, nk, 128], BF16, tag=f"wihm{nk}")
                    nc.sync.dma_start(out=wm[:, :, :], in_=wih_dram[m])
                    wms.append(wm)
                pp = psproj.tile([128, 512], F32, tag="proj")
                nks = sum(s[1] for s in segs)
                kk = 0
                for wm, (_, nk, rhs_of_k, m) in zip(wms, segs):
                    for k in range(nk):
                        nc.tensor.matmul(pp[:, 0:ln],
                                         wm[:, k, :],
                                         rhs_of_k(k, ch)[:, off:off + ln],
                                         start=(kk == 0), stop=(kk == nks - 1))
                        kk += 1
                m0 = segs[0][3]
                base = 512 * ch + off
                nc.vector.tensor_copy(xw[:, m0, base:base + ln],
                                      pp[:, 0:ln])
            return emit

        def xt_rhs(k, ch):
            return xt[:, k, 512 * ch:512 * (ch + 1)]

        # feature rows (delta@p0, mask@p32, ones@p64) in xt[:, 6, :]
        nc.vector.memset(xt[:, 6, :], 0.0)
        nc.vector.memset(xt[64:65, 6, :], 1.0)
        frow = frpool.tile([1, SB], F32, tag="frow", name="frow_r")
        nc.sync.dma_start(out=frow[:, :], in_=rmask[:, :])
        nc.vector.tensor_copy(xt[32:33, 6, :], frow[:, :])
        frow2 = frpool.tile([1, SB], F32, tag="frow", name="frow_d")
        nc.sync.dma_start(out=frow2[:, :], in_=drow[:, :])
        nc.vector.tensor_copy(xt[0:1, 6, :], frow2[:, :])

        # Up front: only row-tile 0 + the first 128 projection columns (16
        # scan steps of lead); the rest of chunk 0 fills the earliest scan
        # gaps. Chunks 1-3 fill the remaining stall gaps.
        for r in range(RPC):
            rowtile_dma(r)
        rowtile_tp(0)
        for m in range(MT):
            proj_item(xw0, 0, [(w["wih0"], K0, xt_rhs, m)], 0, 128)()
        for r in range(1, RPC):
            filler.add(0, r, (lambda rr: lambda: rowtile_tp(rr))(r))
        for m in range(MT):
            filler.add(0, 4 + (10 * m) // MT,
                       proj_item(xw0, 0, [(w["wih0"], K0, xt_rhs, m)],
                                 128, 384))
        for c in range(1, NCH):
            for r in range(RPC * c, RPC * (c + 1)):
                filler.add(SPC * (c - 1), SPC * c - 8,
                           (lambda rr: lambda: rowtile_dma(rr))(r))
                filler.add(SPC * (c - 1), SPC * c - 4,
                           (lambda rr: lambda: rowtile_tp(rr))(r))
            for m in range(MT):
                filler.add(SPC * (c - 1), SPC * c,
                           proj_item(xw0, c, [(w["wih0"], K0, xt_rhs, m)]))

        # The upper half of hdst2 (cols for t >= S/2) is written by scan
        # steps 0..S/2-1, so its AllReduce can run during the scan's second
        # half (both pair cores reach step S/2 at about the same time).
        def exchange_hi():
            nc.sync.dma_start(out=hx["in_hi"].ap(),
                              in_=hdst2[:, :, SB // 2:])
            nc.gpsimd.collective_compute(
                "AllReduce", OP.add, replica_groups=RG,
                ins=[hx["in_hi"].ap().opt()], outs=[hx["out_hi"].ap().opt()])

        filler.add(S // 2, S // 2 + 8, exchange_hi)

        scan_layer(whh0, xw0, hdst, hdst2)
        filler.drain()
        stx.close()
        latepool = st.enter_context(tc.tile_pool(name="late", bufs=1))

        # ---- exchange lower half; hrecv = sum - own ---------------------
        nc.sync.dma_start(out=hx["in_lo"].ap(), in_=hdst2[:, :, :SB // 2])
        nc.gpsimd.collective_compute(
            "AllReduce", OP.add, replica_groups=RG,
            ins=[hx["in_lo"].ap().opt()], outs=[hx["out_lo"].ap().opt()])

        # L1 own-half chunk-0 projection overlaps the collective on the PE
        whh1 = wpool.tile([128, KH, G], BF16, tag="whh", name="whh1")
        nc.sync.dma_start(out=whh1[:, :, :], in_=w["whh1"][:, :, :])
        wo = wpool.tile([128, K1O, C], BF16, tag="wout")
        nc.sync.dma_start(out=wo[:, :, :], in_=w["wout"][:, :, :])

        def own_rhs(k, ch):
            if k < KH:
                return hdst[:, k, 512 * ch:512 * (ch + 1)]
            return ones_row[:, 512 * ch:512 * (ch + 1)]

        def rcv_rhs(k, ch):
            return hrecv[:, k, 512 * ch:512 * (ch + 1)]

        hrecv = latepool.tile([128, KH, SB], BF16, tag="hrecv", name="hrecv")
        nc.sync.dma_start(out=hrecv[:, :, SB // 2:], in_=hx["out_hi"].ap())
        nc.vector.tensor_tensor(hrecv[:, :, SB // 2:],
                                hrecv[:, :, SB // 2:],
                                hdst2[:, :, SB // 2:], OP.subtract)

        xw1 = xwpool.tile([128, MT, SB], BF16, tag="xw", name="xw1")
        for m in range(MT):
            proj_item(xw1, 0, [(w["wih1o"], K1O, own_rhs, m)])()

        nc.sync.dma_start(out=hrecv[:, :, :SB // 2], in_=hx["out_lo"].ap())
        nc.vector.tensor_tensor(hrecv[:, :, :SB // 2],
                                hrecv[:, :, :SB // 2],
                                hdst2[:, :, :SB // 2], OP.subtract)

        def rcv_acc_item(m, ch, off=0, ln=512):
            def emit():
                wm = wpool.tile([128, K1R, 128], BF16, tag=f"wihm{K1R}")
                nc.sync.dma_start(out=wm[:, :, :], in_=w["wih1r"][m])
                pp = psproj.tile([128, 512], F32, tag="proj")
                for k in range(K1R):
                    nc.tensor.matmul(pp[:, 0:ln], wm[:, k, :],
                                     rcv_rhs(k, ch)[:, off:off + ln],
                                     start=(k == 0), stop=(k == K1R - 1))
                base = 512 * ch + off
                nc.vector.tensor_tensor(
                    xw1[:, m, base:base + ln], pp[:, 0:ln],
                    xw1[:, m, base:base + ln], OP.add)
            return emit

        # First 128 recv columns inline (16 scan steps of lead); the rest
        # drains through the filler queue during the first L1 scan steps.
        for m in range(MT):
            rcv_acc_item(m, 0, 0, 128)()
        for m in range(MT):
            filler.add(0, 4 + (10 * m) // MT, rcv_acc_item(m, 0, 128, 384))

        h1 = latepool.tile([128, KH, SB], BF16, tag="h1", name="h1")
        out_sbc = out.rearrange("b s c -> s b c")

        def outproj_item(ch):
            def emit():
                po = psproj.tile([C, 512], F32, tag="proj")
                for k in range(K1O):
                    if k < KH:
                        rhs = h1[:, k, 512 * ch:512 * (ch + 1)]
                    else:
                        rhs = ones_row[:, 512 * ch:512 * (ch + 1)]
                    nc.tensor.matmul(po[:, :], wo[:, k, :], rhs,
                                     start=(k == 0), stop=(k == K1O - 1))
                ost = scpool.tile([C, 512], F32, tag="ost")
                nc.vector.tensor_copy(ost[:, :], po[:, :])
                for cb in range(4):
                    pt = pspool.tile([128, C], F32, tag="tp")
                    nc.tensor.transpose(pt[:, :],
                                        ost[:, 128 * cb:128 * (cb + 1)],
                                        ident[0:C, 0:C])
                    onat = scpool.tile([128, C], F32, tag="onat")
                    nc.vector.tensor_copy(onat[:, :], pt[:, :])
                    gb = 4 * ch + cb
                    nc.sync.dma_start(
                        out=out_sbc[TPR * gb:TPR * (gb + 1), :, :],
                        in_=onat[:, :])
            return emit

        # L1 fillers: remaining xw1 chunks (own+recv 9-chains) trail one
        # chunk ahead of the scan; out-projection chunks trail completion.
        for c in range(1, NCH):
            for m in range(MT):
                filler.add(SPC * (c - 1), SPC * c,
                           proj_item(xw1, c,
                                     [(w["wih1o"], K1O, own_rhs, m),
                                      (w["wih1r"], K1R, rcv_rhs, m)]))
        for c in range(NCH - 1):
            filler.add(SPC * (c + 1), 10 ** 9, outproj_item(c))
        filler.add(10 ** 9, 10 ** 9, outproj_item(NCH - 1))

        scan_layer(whh1, xw1, h1, None)
        filler.drain()


def _get_nc():
    if "nc" not in _cache:
        _cache["nc"] = build_nc()
    return _cache["nc"]


def make_in_maps(inputs):
    wmaps = {d: _prep_weights(inputs, d) for d in ("f", "b")}
    return [_prep_core_inputs(inputs, wmaps, c) for c in range(NCORES)]


def kernel(**inputs):
    from concourse.bass_utils import run_bass_kernel_spmd

    in_maps = make_in_maps(inputs)
    nc = _get_nc()
    res = run_bass_kernel_spmd(nc, in_maps, core_ids=list(range(NCORES)))
    parts = [r["out"] for r in res.results]
    full = np.empty((B, S, C), np.float32)
    for p in range(NPAIR):
        full[BL * p:BL * (p + 1)] = parts[2 * p] + parts[2 * p + 1][:, ::-1]
    return full


# revision 43
# speedup vs baseline: 1.1898x; 1.1898x over previous
"""Trainium2 Bass kernel for nn_ArgumentClassification (2-layer BiLSTM tagger).

Sharding: 8 cores = 4 batch slices x 2 directions. Core c handles batch rows
[c//2*8 : c//2*8+8] and direction ('f' if c%2==0 else 'b'). Backward cores
receive their inputs TIME-REVERSED on the host, so the device program is
identical on every core (pure forward scan); the host un-reverses and sums
the per-direction partial outputs.

This halves the per-core scan weight-load volume vs. batch-only sharding:
the 256-step LSTM recurrence is bound by streaming Whh (2048x512 bf16)
through the PE array every step (~45ns per ldweights+matmul pair), so one
direction per core = 64 pairs/step instead of 128.

Mid-kernel exchange: layer 1 consumes [h0f; h0b]. Each core stores a
time-reversed copy of its layer-0 output (hdst2) -- reversed-in-my-frame is
exactly the peer's time convention -- and the fwd/bwd core pairs AllReduce
their hdst2 through HBM; subtracting one's own contribution leaves the
peer's h0, time-aligned locally. The output projection splits by k:
out = h1f @ Wf.T + (h1b @ Wb.T reversed), summed on the host, so no second
exchange is needed.

Per-core pipeline:
  1. x.T features [128, 7, SB] built from the mean of 4 transformer layers
     (PE transposes), plus delta/mask/ones feature rows. The predicate
     one-hot and role mask are host-precomputed (tiny [B,S] int ops).
  2. L0 input projection (batched matmuls, bias folded via ones-row).
  3. L0 scan: 256 steps in gates-transposed layout [2048, BL], Whh
     stationary, gate groups in (g,i,f,o) order so the DVE/ACT nonlinearity
     chain of each group overlaps the next group's matmuls.
  4. hdst2 AllReduce with pair core; hrecv = sum - hdst2.
  5. L1 projection from [hdst(own); hrecv(peer)] + bias, L1 scan.
  6. out partial = h1 @ W_out[own half].T (+ bias on fwd cores only),
     PE-transposed to [BL, S, 30] and DMA'd out.

Gate order is host-permuted from PyTorch's (i,f,g,o) to (i,f,o,g).
"""
import sys

sys.path.insert(0, "/opt/trn_rl_repo")

import numpy as np
import ml_dtypes

import concourse.bass as bass
import concourse.tile as tile
from concourse import bacc, mybir
from concourse.bass import ds
from concourse.masks import make_identity

BF16 = mybir.dt.bfloat16
F32 = mybir.dt.float32
AF = mybir.ActivationFunctionType
OP = mybir.AluOpType

B, S, E, H, C = 32, 256, 768, 512, 30
NCORES = 8
NPAIR = 4                 # batch slices (pairs of cores)
BL = B // NPAIR           # 8 rows per core
SB = S * BL               # 2048 columns, ordered (t, b): col = t*BL + b
G = 4 * H                 # 2048 gate rows
MT = G // 128             # 16 gate m-tiles
KH = H // 128             # 4 hidden k-tiles
K0 = 7                    # L0 input k-tiles ([770 + ones-row] padded to 896)
K1O = 5                   # L1 own-half k-tiles (512 + bias row -> 640)
K1R = 4                   # L1 recv-half k-tiles (512)
RG = [[0, 1], [2, 3], [4, 5], [6, 7]]

_cache = {}


def _bf(a):
    return np.asarray(a, dtype=ml_dtypes.bfloat16)


def _prep_weights(inp, d):
    """Host-side weight prep for direction d ('f'/'b'): permute gates to
    (i,f,o,g), transpose, pad, fold biases, tile for SBUF."""
    perm = np.concatenate([
        np.arange(0, H),          # i
        np.arange(H, 2 * H),      # f
        np.arange(3 * H, 4 * H),  # o
        np.arange(2 * H, 3 * H),  # g
    ])
    out = {}

    def tile_k(a, nk):
        # [nk*128, M] -> [128, nk, M]
        return np.ascontiguousarray(
            a.reshape(nk, 128, a.shape[1]).transpose(1, 0, 2))

    def tile_km(a, nk):
        # [nk*128, 16*128] -> [16, 128, nk, 128]  (per-m-block contiguous)
        m = a.shape[1] // 128
        return np.ascontiguousarray(
            a.reshape(nk, 128, m, 128).transpose(2, 1, 0, 3))

    own = slice(0, H) if d == "f" else slice(H, 2 * H)
    rcv = slice(H, 2 * H) if d == "f" else slice(0, H)

    # layer 0
    wih = inp[f"Wih_l0{d}"][perm]                     # [2048, 770]
    bias = (inp[f"bih_l0{d}"] + inp[f"bhh_l0{d}"])[perm]
    ext = np.zeros((K0 * 128, G), np.float32)
    ext[:768] = wih.T[:768]
    ext[768] = wih.T[768]      # delta coeffs at tile6 partition 0
    ext[800] = wih.T[769]      # mask coeffs at tile6 partition 32
    ext[832] = bias            # bias row at tile6 partition 64
    out["wih0"] = _bf(tile_km(ext, K0))               # [16,128,7,128]
    whh = inp[f"Whh_l0{d}"][perm]                     # [2048, 512]
    out["whh0"] = _bf(tile_k(whh.T, KH))              # [128, 4, 2048]

    # layer 1, split into own-half (+bias) and recv-half
    w1T = inp[f"Wih_l1{d}"][perm].T                   # [1024, 2048]
    bias = (inp[f"bih_l1{d}"] + inp[f"bhh_l1{d}"])[perm]
    ext = np.zeros((K1O * 128, G), np.float32)
    ext[:512] = w1T[own]
    ext[512] = bias
    out["wih1o"] = _bf(tile_km(ext, K1O))             # [16,128,5,128]
    ext = np.zeros((K1R * 128, G), np.float32)
    ext[:512] = w1T[rcv]
    out["wih1r"] = _bf(tile_km(ext, K1R))             # [16,128,4,128]
    whh = inp[f"Whh_l1{d}"][perm]
    out["whh1"] = _bf(tile_k(whh.T, KH))

    # output projection own half [640, 30]; bias only on fwd cores
    ext = np.zeros((K1O * 128, C), np.float32)
    ext[:512] = inp["W_out"].T[own]
    if d == "f":
        ext[512] = inp["b_out"]
    out["wout"] = _bf(tile_k(ext, K1O))               # [128, 5, 30]
    return out


def _prep_core_inputs(inputs, wmaps, core):
    pair, parity = core // 2, core % 2
    d = "f" if parity == 0 else "b"
    rows = slice(BL * pair, BL * (pair + 1))

    hs = np.asarray(inputs["hidden_states"], np.float32)[:, rows]  # [4,BL,S,E]
    roles = np.asarray(inputs["roles"])[rows]                      # [BL,S]
    preds = np.asarray(inputs["predicates"])[rows]
    rmask = ((roles != 0) & (roles != -100)).astype(np.float32)
    idx = np.argmax(preds, axis=-1)                                # [BL]
    mw = hs.mean(axis=0).mean(axis=-1)                             # [BL,S]
    delta = (mw - np.take_along_axis(mw, idx[:, None], 1)).astype(np.float32)
    if parity == 1:  # time-reverse for backward cores
        hs = hs[:, :, ::-1]
        rmask = rmask[:, ::-1]
        delta = delta[:, ::-1]
    m = dict(wmaps[d])
    m["hs"] = _bf(hs)
    m["rmask"] = np.ascontiguousarray(rmask.T).reshape(1, SB)      # (t,b)
    m["drow"] = np.ascontiguousarray(delta.T).reshape(1, SB)
    return m


def build_nc():
    nc = bacc.Bacc("TRN2", target_bir_lowering=False, debug=False,
                   num_devices=NCORES)
    hs = nc.dram_tensor("hs", [4, BL, S, E], BF16, kind="ExternalInput").ap()
    rmask = nc.dram_tensor("rmask", [1, SB], F32, kind="ExternalInput").ap()
    drow = nc.dram_tensor("drow", [1, SB], F32, kind="ExternalInput").ap()
    w = {}
    w["wih0"] = nc.dram_tensor("wih0", [MT, 128, K0, 128], BF16,
                               kind="ExternalInput").ap()
    w["wih1o"] = nc.dram_tensor("wih1o", [MT, 128, K1O, 128], BF16,
                                kind="ExternalInput").ap()
    w["wih1r"] = nc.dram_tensor("wih1r", [MT, 128, K1R, 128], BF16,
                                kind="ExternalInput").ap()
    w["whh0"] = nc.dram_tensor("whh0", [128, KH, G], BF16,
                               kind="ExternalInput").ap()
    w["whh1"] = nc.dram_tensor("whh1", [128, KH, G], BF16,
                               kind="ExternalInput").ap()
    w["wout"] = nc.dram_tensor("wout", [128, K1O, C], BF16,
                               kind="ExternalInput").ap()
    hx = {}
    for half in ("hi", "lo"):
        hx[f"in_{half}"] = nc.dram_tensor(f"hx_in_{half}", [128, KH, SB // 2],
                                          BF16, kind="Internal")
        hx[f"out_{half}"] = nc.dram_tensor(f"hx_out_{half}",
                                           [128, KH, SB // 2], BF16,
                                           kind="Internal")
    out = nc.dram_tensor("out", [BL, S, C], F32, kind="ExternalOutput").ap()

    with tile.TileContext(nc) as tc:
        _emit(nc, tc, hs, rmask, drow, w, hx, out)
    nc.compile()
    return nc


class _Filler:
    """Deadline-aware FIFO of emission thunks. Items are emitted between
    scan steps so their PE work lands in the scan's dependency-stall gaps.
    Strict FIFO pops keep PE program order consistent with producer ->
    consumer order (no in-order-engine deadlocks)."""

    def __init__(self):
        self.q = []

    def add(self, earliest, deadline, fn):
        self.q.append((earliest, deadline, fn))

    def step(self, j, budget=1):
        n = 0
        while self.q and self.q[0][1] <= j:
            self.q.pop(0)[2]()
            n += 1
        while self.q and n < budget and self.q[0][0] <= j:
            self.q.pop(0)[2]()
            n += 1

    def drain(self):
        while self.q:
            self.q.pop(0)[2]()


def _emit(nc, tc, hs, rmask, drow, w, hx, out):
    from contextlib import ExitStack
    NCH = SB // 512          # 4 column chunks (64 timesteps each)
    SPC = S // NCH           # 64 scan steps per chunk
    NRT = SB // 128          # 16 (t,b) row-tiles
    RPC = NRT // NCH         # 4 row-tiles per chunk
    TPR = 128 // BL          # 16 timesteps per row-tile
    with ExitStack() as st:
        cpool = st.enter_context(tc.tile_pool(name="const", bufs=1))
        rpool = st.enter_context(tc.tile_pool(name="rows", bufs=1))
        xwpool = st.enter_context(tc.tile_pool(name="xw", bufs=1))
        scpool = st.enter_context(tc.tile_pool(name="sc", bufs=3))
        wpool = st.enter_context(tc.tile_pool(name="wts", bufs=2))
        pspool = st.enter_context(tc.tile_pool(name="ps", bufs=1, space="PSUM"))
        psproj = st.enter_context(tc.tile_pool(name="psp", bufs=2,
                                               space="PSUM"))
        psg = st.enter_context(tc.tile_pool(name="psg", bufs=5, space="PSUM"))

        ident = cpool.tile([128, 128], F32, tag="ident")
        make_identity(nc, ident[:, :])
        ones_row = cpool.tile([128, SB], BF16, tag="onesrow")
        nc.vector.memset(ones_row[:, :], 0.0)
        nc.vector.memset(ones_row[0:1, :], 1.0)

        whh0 = wpool.tile([128, KH, G], BF16, tag="whh", name="whh0")
        nc.sync.dma_start(out=whh0[:, :, :], in_=w["whh0"][:, :, :])
        hdst = rpool.tile([128, KH, SB], BF16, tag="hdst", name="hdst")
        hdst2 = rpool.tile([128, KH, SB], BF16, tag="hdst2", name="hdst2")

        filler = _Filler()

        def scan_layer(whh_sb, xw, hd, hd2):
            """Single-direction 256-step scan with gap-filler items."""
            hbuf = rpool.tile([128, 2, KH, BL], BF16, tag="hbuf", name="hbuf")
            nc.vector.memset(hbuf[:, 0, :, :], 0.0)
            cbuf = rpool.tile([128, KH, BL], F32, tag="cbuf", name="cbuf")
            nc.vector.memset(cbuf[:, :, :], 0.0)

            for j in range(S):
                filler.step(j)
                cur, nxt = j % 2, (j + 1) % 2
                cols = j * BL
                gorder = (3, 0, 1, 2)          # g, i, f, o
                pg, gs = {}, {}
                for gg in gorder:
                    pg[gg] = psg.tile([128, KH, BL], F32, tag="gates",
                                      name=f"pg{gg}")
                    for mm in range(KH):
                        m = 4 * gg + mm
                        for k in range(KH):
                            nc.tensor.matmul(
                                pg[gg][:, mm, :],
                                whh_sb[:, k, 128 * m:128 * (m + 1)],
                                hbuf[:, cur, k, :],
                                start=(k == 0), stop=(k == KH - 1))
                    gs[gg] = scpool.tile([128, KH, BL], F32, tag=f"gs{gg}",
                                         name=f"gs{gg}")
                    nc.vector.tensor_tensor(
                        gs[gg][:, :, :], pg[gg][:, :, :],
                        xw[:, 4 * gg:4 * gg + 4, ds(cols, BL)], OP.add)
                    if gg == 3:
                        tg = scpool.tile([128, KH, BL], F32, tag="tg",
                                         name="tg")
                        nc.scalar.activation(tg[:, :, :], gs[3][:, :, :],
                                             AF.Tanh)
                    elif gg == 0:
                        si = scpool.tile([128, KH, BL], F32, tag="si")
                        nc.scalar.activation(si[:, :, :], gs[0][:, :, :],
                                             AF.Sigmoid)
                        t1 = scpool.tile([128, KH, BL], F32, tag="t1",
                                         name="t1")
                        nc.vector.tensor_tensor(t1[:, :, :], si[:, :, :],
                                                tg[:, :, :], OP.mult)
                    elif gg == 1:
                        sf = scpool.tile([128, KH, BL], F32, tag="sf")
                        nc.scalar.activation(sf[:, :, :], gs[1][:, :, :],
                                             AF.Sigmoid)
                        t2 = scpool.tile([128, KH, BL], F32, tag="t2")
                        nc.vector.tensor_tensor(t2[:, :, :], sf[:, :, :],
                                                cbuf[:, :, :], OP.mult)
                        nc.vector.tensor_tensor(cbuf[:, :, :], t1[:, :, :],
                                                t2[:, :, :], OP.add)
                        tcc = scpool.tile([128, KH, BL], F32, tag="tcc",
                                          name="tcc")
                        nc.scalar.activation(tcc[:, :, :], cbuf[:, :, :],
                                             AF.Tanh)
                    else:
                        so = scpool.tile([128, KH, BL], F32, tag="so",
                                         name="so")
                        nc.scalar.activation(so[:, :, :], gs[2][:, :, :],
                                             AF.Sigmoid)
                        nc.vector.tensor_tensor(hbuf[:, nxt, :, :],
                                                so[:, :, :], tcc[:, :, :],
                                                OP.mult)
                nc.vector.tensor_tensor(hd[:, :, ds(cols, BL)],
                                        so[:, :, :], tcc[:, :, :], OP.mult)
                if hd2 is not None:
                    nc.vector.tensor_tensor(
                        hd2[:, :, ds((S - 1 - j) * BL, BL)],
                        so[:, :, :], tcc[:, :, :], OP.mult)

        # ---- layer 0: x.T features + projection, chunk-pipelined ---------
        xw0 = xwpool.tile([128, MT, SB], BF16, tag="xw", name="xw0")
        stx = st.enter_context(ExitStack())
        xtpool = stx.enter_context(tc.tile_pool(name="xtp", bufs=1))
        hlpool = stx.enter_context(tc.tile_pool(name="hl", bufs=5))
        sumpool = stx.enter_context(tc.tile_pool(name="sum", bufs=4))
        frpool = stx.enter_context(tc.tile_pool(name="frp", bufs=1))

        xt = xtpool.tile([128, K0, SB], BF16, tag="xt")
        hs_sbe = hs.rearrange("l b s e -> l s b e")
        rt_sum = {}

        def rowtile_dma(r):
            hl = []
            for layer in range(4):
                t = hlpool.tile([128, E], BF16, tag="hl")
                nc.sync.dma_start(
                    out=t[:, :],
                    in_=hs_sbe[layer, TPR * r:TPR * (r + 1), :, :])
                hl.append(t)
            s01 = sumpool.tile([128, E], F32, tag="sum")
            nc.vector.tensor_tensor(s01[:, :], hl[0][:, :], hl[1][:, :],
                                    OP.add)
            s23 = sumpool.tile([128, E], F32, tag="sum")
            nc.vector.tensor_tensor(s23[:, :], hl[2][:, :], hl[3][:, :],
                                    OP.add)
            ssum = sumpool.tile([128, E], F32, tag="sum")
            nc.vector.tensor_tensor(ssum[:, :], s01[:, :], s23[:, :], OP.add)
            rt_sum[r] = ssum

        def rowtile_tp(r):
            ssum = rt_sum.pop(r)
            for c in range(6):
                pt = pspool.tile([128, 128], F32, tag="tp")
                nc.tensor.transpose(pt[:, :], ssum[:, 128 * c:128 * (c + 1)],
                                    ident[:, :])
                nc.vector.tensor_scalar_mul(
                    xt[:, c, 128 * r:128 * (r + 1)], pt[:, :], 0.25)

        def proj_item(xw, ch, segs, off=0, ln=512):
            """One m-tile, one column sub-range of a 512-col chunk: weight
            DMAs + one accumulation chain + PSUM->SBUF copy. segs: list of
            (wih_dram, nk, rhs_of_k, m). Returns a thunk."""
            def emit():
                wms = []
                for (wih_dram, nk, _, m) in segs:
                    wm = wpool.tile([128, nk, 128], BF16, tag=f"wihm{nk}")
                    nc.sync.dma_start(out=wm[:, :, :], in_=wih_dram[m])
                    wms.append(wm)
                pp = psproj.tile([128, 512], F32, tag="proj")
                nks = sum(s[1] for s in segs)
                kk = 0
                for wm, (_, nk, rhs_of_k, m) in zip(wms, segs):
                    for k in range(nk):
                        nc.tensor.matmul(pp[:, 0:ln],
                                         wm[:, k, :],
                                         rhs_of_k(k, ch)[:, off:off + ln],
                                         start=(kk == 0), stop=(kk == nks - 1))
                        kk += 1
                m0 = segs[0][3]
                base = 512 * ch + off
                nc.vector.tensor_copy(xw[:, m0, base:base + ln],
                                      pp[:, 0:ln])
            return emit

        def xt_rhs(k, ch):
            return xt[:, k, 512 * ch:512 * (ch + 1)]

        # feature rows (delta@p0, mask@p32, ones@p64) in xt[:, 6, :]
        nc.vector.memset(xt[:, 6, :], 0.0)
        nc.vector.memset(xt[64:65, 6, :], 1.0)
        frow = frpool.tile([1, SB], F32, tag="frow", name="frow_r")
        nc.sync.dma_start(out=frow[:, :], in_=rmask[:, :])
        nc.vector.tensor_copy(xt[32:33, 6, :], frow[:, :])
        frow2 = frpool.tile([1, SB], F32, tag="frow", name="frow_d")
        nc.sync.dma_start(out=frow2[:, :], in_=drow[:, :])
        nc.vector.tensor_copy(xt[0:1, 6, :], frow2[:, :])

        # Up front: only row-tile 0 + the first 128 projection columns (16
        # scan steps of lead); the rest of chunk 0 fills the earliest scan
        # gaps. Chunks 1-3 fill the remaining stall gaps.
        for r in range(RPC):
            rowtile_dma(r)
        rowtile_tp(0)
        for m in range(MT):
            proj_item(xw0, 0, [(w["wih0"], K0, xt_rhs, m)], 0, 128)()
        for r in range(1, RPC):
            filler.add(0, r, (lambda rr: lambda: rowtile_tp(rr))(r))
        for m in range(MT):
            filler.add(0, 4 + (10 * m) // MT,
                       proj_item(xw0, 0, [(w["wih0"], K0, xt_rhs, m)],
                                 128, 384))
        for c in range(1, NCH):
            for r in range(RPC * c, RPC * (c + 1)):
                filler.add(SPC * (c - 1), SPC * c - 8,
                           (lambda rr: lambda: rowtile_dma(rr))(r))
                filler.add(SPC * (c - 1), SPC * c - 4,
                           (lambda rr: lambda: rowtile_tp(rr))(r))
            for m in range(MT):
                filler.add(SPC * (c - 1), SPC * c,
                           proj_item(xw0, c, [(w["wih0"], K0, xt_rhs, m)]))

        # The upper half of hdst2 (cols for t >= S/2) is written by scan
        # steps 0..S/2-1, so its AllReduce can run during the scan's second
        # half (both pair cores reach step S/2 at about the same time).
        def exchange_hi():
            nc.sync.dma_start(out=hx["in_hi"].ap(),
                              in_=hdst2[:, :, SB // 2:])
            nc.gpsimd.collective_compute(
                "AllReduce", OP.add, replica_groups=RG,
                ins=[hx["in_hi"].ap().opt()], outs=[hx["out_hi"].ap().opt()])

        filler.add(S // 2, S // 2 + 8, exchange_hi)

        scan_layer(whh0, xw0, hdst, hdst2)
        filler.drain()
        stx.close()
        latepool = st.enter_context(tc.tile_pool(name="late", bufs=1))

        # ---- exchange lower half; hrecv = sum - own ---------------------
        nc.sync.dma_start(out=hx["in_lo"].ap(), in_=hdst2[:, :, :SB // 2])
        nc.gpsimd.collective_compute(
            "AllReduce", OP.add, replica_groups=RG,
            ins=[hx["in_lo"].ap().opt()], outs=[hx["out_lo"].ap().opt()])

        # L1 own-half chunk-0 projection overlaps the collective on the PE
        whh1 = wpool.tile([128, KH, G], BF16, tag="whh", name="whh1")
        nc.sync.dma_start(out=whh1[:, :, :], in_=w["whh1"][:, :, :])
        wo = wpool.tile([128, K1O, C], BF16, tag="wout")
        nc.sync.dma_start(out=wo[:, :, :], in_=w["wout"][:, :, :])

        def own_rhs(k, ch):
            if k < KH:
                return hdst[:, k, 512 * ch:512 * (ch + 1)]
            return ones_row[:, 512 * ch:512 * (ch + 1)]

        def rcv_rhs(k, ch):
            return hrecv[:, k, 512 * ch:512 * (ch + 1)]

        hrecv = latepool.tile([128, KH, SB], BF16, tag="hrecv", name="hrecv")
        nc.sync.dma_start(out=hrecv[:, :, SB // 2:], in_=hx["out_hi"].ap())
        nc.vector.tensor_tensor(hrecv[:, :, SB // 2:],
                                hrecv[:, :, SB // 2:],
                                hdst2[:, :, SB // 2:], OP.subtract)

        xw1 = xwpool.tile([128, MT, SB], BF16, tag="xw", name="xw1")
        for m in range(MT):
            proj_item(xw1, 0, [(w["wih1o"], K1O, own_rhs, m)])()

        nc.sync.dma_start(out=hrecv[:, :, :SB // 2], in_=hx["out_lo"].ap())
        nc.vector.tensor_tensor(hrecv[:, :, :SB // 2],
                                hrecv[:, :, :SB // 2],
                                hdst2[:, :, :SB // 2], OP.subtract)

        def rcv_acc_item(m, ch, off=0, ln=512):
            def emit():
                wm = wpool.tile([128, K1R, 128], BF16, tag=f"wihm{K1R}")
                nc.sync.dma_start(out=wm[:, :, :], in_=w["wih1r"][m])
                pp = psproj.tile([128, 512], F32, tag="proj")
                for k in range(K1R):
                    nc.tensor.matmul(pp[:, 0:ln], wm[:, k, :],
                                     rcv_rhs(k, ch)[:, off:off + ln],
                                     start=(k == 0), stop=(k == K1R - 1))
                base = 512 * ch + off
                nc.vector.tensor_tensor(
                    xw1[:, m, base:base + ln], pp[:, 0:ln],
                    xw1[:, m, base:base + ln], OP.add)
            return emit

        # First 128 recv columns inline (16 L1 scan steps of lead); the
        # remainder drains through the filler queue in the early scan gaps.
        for m in range(MT):
            rcv_acc_item(m, 0, 0, 128)()
        for m in range(MT):
            filler.add(0, 4 + (10 * m) // MT, rcv_acc_item(m, 0, 128, 384))

        h1 = latepool.tile([128, KH, SB], BF16, tag="h1", name="h1")
        out_sbc = out.rearrange("b s c -> s b c")

        def outproj_item(ch):
            def emit():
                po = psproj.tile([C, 512], F32, tag="proj")
                for k in range(K1O):
                    if k < KH:
                        rhs = h1[:, k, 512 * ch:512 * (ch + 1)]
                    else:
                        rhs = ones_row[:, 512 * ch:512 * (ch + 1)]
                    nc.tensor.matmul(po[:, :], wo[:, k, :], rhs,
                                     start=(k == 0), stop=(k == K1O - 1))
                ost = scpool.tile([C, 512], F32, tag="ost")
                nc.vector.tensor_copy(ost[:, :], po[:, :])
                for cb in range(4):
                    pt = pspool.tile([128, C], F32, tag="tp")
                    nc.tensor.transpose(pt[:, :],
                                        ost[:, 128 * cb:128 * (cb + 1)],
                                        ident[0:C, 0:C])
                    onat = scpool.tile([128, C], F32, tag="onat")
                    nc.vector.tensor_copy(onat[:, :], pt[:, :])
                    gb = 4 * ch + cb
                    nc.sync.dma_start(
                        out=out_sbc[TPR * gb:TPR * (gb + 1), :, :],
                        in_=onat[:, :])
            return emit

        # L1 fillers: remaining xw1 chunks (own+recv 9-chains) trail one
        # chunk ahead of the scan; out-projection chunks trail completion.
        for c in range(1, NCH):
            for m in range(MT):
                filler.add(SPC * (c - 1), SPC * c,
                           proj_item(xw1, c,
                                     [(w["wih1o"], K1O, own_rhs, m),
                                      (w["wih1r"], K1R, rcv_rhs, m)]))
        for c in range(NCH - 1):
            filler.add(SPC * (c + 1), 10 ** 9, outproj_item(c))
        filler.add(10 ** 9, 10 ** 9, outproj_item(NCH - 1))

        scan_layer(whh1, xw1, h1, None)
        filler.drain()


def _get_nc():
    if "nc" not in _cache:
        _cache["nc"] = build_nc()
    return _cache["nc"]


def make_in_maps(inputs):
    wmaps = {d: _prep_weights(inputs, d) for d in ("f", "b")}
    return [_prep_core_inputs(inputs, wmaps, c) for c in range(NCORES)]


def kernel(**inputs):
    from concourse.bass_utils import run_bass_kernel_spmd

    in_maps = make_in_maps(inputs)
    nc = _get_nc()
    res = run_bass_kernel_spmd(nc, in_maps, core_ids=list(range(NCORES)))
    parts = [r["out"] for r in res.results]
    full = np.empty((B, S, C), np.float32)
    for p in range(NPAIR):
        full[BL * p:BL * (p + 1)] = parts[2 * p] + parts[2 * p + 1][:, ::-1]
    return full


# revision 46
# speedup vs baseline: 1.1967x; 1.0059x over previous
"""Trainium2 Bass kernel for nn_ArgumentClassification (2-layer BiLSTM tagger).

Sharding: 8 cores = 4 batch slices x 2 directions. Core c handles batch rows
[c//2*8 : c//2*8+8] and direction ('f' if c%2==0 else 'b'). Backward cores
receive their inputs TIME-REVERSED on the host, so the device program is
identical on every core (pure forward scan); the host un-reverses and sums
the per-direction partial outputs.

This halves the per-core scan weight-load volume vs. batch-only sharding:
the 256-step LSTM recurrence is bound by streaming Whh (2048x512 bf16)
through the PE array every step (~45ns per ldweights+matmul pair), so one
direction per core = 64 pairs/step instead of 128.

Mid-kernel exchange: layer 1 consumes [h0f; h0b]. Each core stores a
time-reversed copy of its layer-0 output (hdst2) -- reversed-in-my-frame is
exactly the peer's time convention -- and the fwd/bwd core pairs AllReduce
their hdst2 through HBM; subtracting one's own contribution leaves the
peer's h0, time-aligned locally. The output projection splits by k:
out = h1f @ Wf.T + (h1b @ Wb.T reversed), summed on the host, so no second
exchange is needed.

Per-core pipeline:
  1. x.T features [128, 7, SB] built from the mean of 4 transformer layers
     (PE transposes), plus delta/mask/ones feature rows. The predicate
     one-hot and role mask are host-precomputed (tiny [B,S] int ops).
  2. L0 input projection (batched matmuls, bias folded via ones-row).
  3. L0 scan: 256 steps in gates-transposed layout [2048, BL], Whh
     stationary, gate groups in (g,i,f,o) order so the DVE/ACT nonlinearity
     chain of each group overlaps the next group's matmuls.
  4. hdst2 AllReduce with pair core; hrecv = sum - hdst2.
  5. L1 projection from [hdst(own); hrecv(peer)] + bias, L1 scan.
  6. out partial = h1 @ W_out[own half].T (+ bias on fwd cores only),
     PE-transposed to [BL, S, 30] and DMA'd out.

Gate order is host-permuted from PyTorch's (i,f,g,o) to (i,f,o,g).
"""
import sys

sys.path.insert(0, "/opt/trn_rl_repo")

import numpy as np
import ml_dtypes

import concourse.bass as bass
import concourse.tile as tile
from concourse import bacc, mybir
from concourse.bass import ds
from concourse.masks import make_identity

BF16 = mybir.dt.bfloat16
F32 = mybir.dt.float32
AF = mybir.ActivationFunctionType
OP = mybir.AluOpType

B, S, E, H, C = 32, 256, 768, 512, 30
NCORES = 8
NPAIR = 4                 # batch slices (pairs of cores)
BL = B // NPAIR           # 8 rows per core
SB = S * BL               # 2048 columns, ordered (t, b): col = t*BL + b
G = 4 * H                 # 2048 gate rows
MT = G // 128             # 16 gate m-tiles
KH = H // 128             # 4 hidden k-tiles
K0 = 7                    # L0 input k-tiles ([770 + ones-row] padded to 896)
K1O = 5                   # L1 own-half k-tiles (512 + bias row -> 640)
K1R = 4                   # L1 recv-half k-tiles (512)
RG = [[0, 1], [2, 3], [4, 5], [6, 7]]

_cache = {}


def _bf(a):
    return np.asarray(a, dtype=ml_dtypes.bfloat16)


def _prep_weights(inp, d):
    """Host-side weight prep for direction d ('f'/'b'): permute gates to
    (i,f,o,g), transpose, pad, fold biases, tile for SBUF."""
    perm = np.concatenate([
        np.arange(0, H),          # i
        np.arange(H, 2 * H),      # f
        np.arange(3 * H, 4 * H),  # o
        np.arange(2 * H, 3 * H),  # g
    ])
    out = {}

    def tile_k(a, nk):
        # [nk*128, M] -> [128, nk, M]
        return np.ascontiguousarray(
            a.reshape(nk, 128, a.shape[1]).transpose(1, 0, 2))

    def tile_km(a, nk):
        # [nk*128, 16*128] -> [16, 128, nk, 128]  (per-m-block contiguous)
        m = a.shape[1] // 128
        return np.ascontiguousarray(
            a.reshape(nk, 128, m, 128).transpose(2, 1, 0, 3))

    own = slice(0, H) if d == "f" else slice(H, 2 * H)
    rcv = slice(H, 2 * H) if d == "f" else slice(0, H)

    # layer 0
    wih = inp[f"Wih_l0{d}"][perm]                     # [2048, 770]
    bias = (inp[f"bih_l0{d}"] + inp[f"bhh_l0{d}"])[perm]
    ext = np.zeros((K0 * 128, G), np.float32)
    ext[:768] = wih.T[:768]
    ext[768] = wih.T[768]      # delta coeffs at tile6 partition 0
    ext[800] = wih.T[769]      # mask coeffs at tile6 partition 32
    ext[832] = bias            # bias row at tile6 partition 64
    out["wih0"] = _bf(tile_km(ext, K0))               # [16,128,7,128]
    whh = inp[f"Whh_l0{d}"][perm]                     # [2048, 512]
    out["whh0"] = _bf(tile_k(whh.T, KH))              # [128, 4, 2048]

    # Layer 1 runs the OPPOSITE direction on this core (the scan walks its
    # local columns backwards, which consumes the exchanged peer-h0 halves
    # in the order the split collectives produce them). "own" still refers
    # to this core's L0 output h0{d}.
    d1 = "b" if d == "f" else "f"
    w1T = inp[f"Wih_l1{d1}"][perm].T                  # [1024, 2048]
    bias = (inp[f"bih_l1{d1}"] + inp[f"bhh_l1{d1}"])[perm]
    ext = np.zeros((K1O * 128, G), np.float32)
    ext[:512] = w1T[own]
    ext[512] = bias
    out["wih1o"] = _bf(tile_km(ext, K1O))             # [16,128,5,128]
    ext = np.zeros((K1R * 128, G), np.float32)
    ext[:512] = w1T[rcv]
    out["wih1r"] = _bf(tile_km(ext, K1R))             # [16,128,4,128]
    whh = inp[f"Whh_l1{d1}"][perm]
    out["whh1"] = _bf(tile_k(whh.T, KH))

    # output projection: the W_out half matching h1{d1}; bias on even cores
    oh = slice(0, H) if d1 == "f" else slice(H, 2 * H)
    ext = np.zeros((K1O * 128, C), np.float32)
    ext[:512] = inp["W_out"].T[oh]
    if d == "f":
        ext[512] = inp["b_out"]
    out["wout"] = _bf(tile_k(ext, K1O))               # [128, 5, 30]
    return out


def _prep_core_inputs(inputs, wmaps, core):
    pair, parity = core // 2, core % 2
    d = "f" if parity == 0 else "b"
    rows = slice(BL * pair, BL * (pair + 1))

    hs = np.asarray(inputs["hidden_states"], np.float32)[:, rows]  # [4,BL,S,E]
    roles = np.asarray(inputs["roles"])[rows]                      # [BL,S]
    preds = np.asarray(inputs["predicates"])[rows]
    rmask = ((roles != 0) & (roles != -100)).astype(np.float32)
    idx = np.argmax(preds, axis=-1)                                # [BL]
    mw = hs.mean(axis=0).mean(axis=-1)                             # [BL,S]
    delta = (mw - np.take_along_axis(mw, idx[:, None], 1)).astype(np.float32)
    if parity == 1:  # time-reverse for backward cores
        hs = hs[:, :, ::-1]
        rmask = rmask[:, ::-1]
        delta = delta[:, ::-1]
    m = dict(wmaps[d])
    m["hs"] = _bf(hs)
    m["rmask"] = np.ascontiguousarray(rmask.T).reshape(1, SB)      # (t,b)
    m["drow"] = np.ascontiguousarray(delta.T).reshape(1, SB)
    return m


def build_nc():
    nc = bacc.Bacc("TRN2", target_bir_lowering=False, debug=False,
                   num_devices=NCORES)
    hs = nc.dram_tensor("hs", [4, BL, S, E], BF16, kind="ExternalInput").ap()
    rmask = nc.dram_tensor("rmask", [1, SB], F32, kind="ExternalInput").ap()
    drow = nc.dram_tensor("drow", [1, SB], F32, kind="ExternalInput").ap()
    w = {}
    w["wih0"] = nc.dram_tensor("wih0", [MT, 128, K0, 128], BF16,
                               kind="ExternalInput").ap()
    w["wih1o"] = nc.dram_tensor("wih1o", [MT, 128, K1O, 128], BF16,
                                kind="ExternalInput").ap()
    w["wih1r"] = nc.dram_tensor("wih1r", [MT, 128, K1R, 128], BF16,
                                kind="ExternalInput").ap()
    w["whh0"] = nc.dram_tensor("whh0", [128, KH, G], BF16,
                               kind="ExternalInput").ap()
    w["whh1"] = nc.dram_tensor("whh1", [128, KH, G], BF16,
                               kind="ExternalInput").ap()
    w["wout"] = nc.dram_tensor("wout", [128, K1O, C], BF16,
                               kind="ExternalInput").ap()
    hx = {}
    for half in ("hi", "lo"):
        hx[f"in_{half}"] = nc.dram_tensor(f"hx_in_{half}", [128, KH, SB // 2],
                                          BF16, kind="Internal")
        hx[f"out_{half}"] = nc.dram_tensor(f"hx_out_{half}",
                                           [128, KH, SB // 2], BF16,
                                           kind="Internal")
    out = nc.dram_tensor("out", [BL, S, C], F32, kind="ExternalOutput").ap()

    with tile.TileContext(nc) as tc:
        _emit(nc, tc, hs, rmask, drow, w, hx, out)
    nc.compile()
    return nc


class _Filler:
    """Deadline-aware FIFO of emission thunks. Items are emitted between
    scan steps so their PE work lands in the scan's dependency-stall gaps.
    Strict FIFO pops keep PE program order consistent with producer ->
    consumer order (no in-order-engine deadlocks)."""

    def __init__(self):
        self.q = []

    def add(self, earliest, deadline, fn):
        self.q.append((earliest, deadline, fn))

    def step(self, j, budget=1):
        n = 0
        while self.q and self.q[0][1] <= j:
            self.q.pop(0)[2]()
            n += 1
        while self.q and n < budget and self.q[0][0] <= j:
            self.q.pop(0)[2]()
            n += 1

    def drain(self):
        while self.q:
            self.q.pop(0)[2]()


def _emit(nc, tc, hs, rmask, drow, w, hx, out):
    from contextlib import ExitStack
    NCH = SB // 512          # 4 column chunks (64 timesteps each)
    SPC = S // NCH           # 64 scan steps per chunk
    NRT = SB // 128          # 16 (t,b) row-tiles
    RPC = NRT // NCH         # 4 row-tiles per chunk
    TPR = 128 // BL          # 16 timesteps per row-tile
    with ExitStack() as st:
        cpool = st.enter_context(tc.tile_pool(name="const", bufs=1))
        rpool = st.enter_context(tc.tile_pool(name="rows", bufs=1))
        xwpool = st.enter_context(tc.tile_pool(name="xw", bufs=1))
        scpool = st.enter_context(tc.tile_pool(name="sc", bufs=3))
        wpool = st.enter_context(tc.tile_pool(name="wts", bufs=2))
        pspool = st.enter_context(tc.tile_pool(name="ps", bufs=1, space="PSUM"))
        psproj = st.enter_context(tc.tile_pool(name="psp", bufs=2,
                                               space="PSUM"))
        psg = st.enter_context(tc.tile_pool(name="psg", bufs=5, space="PSUM"))

        ident = cpool.tile([128, 128], F32, tag="ident")
        make_identity(nc, ident[:, :])
        ones_row = cpool.tile([128, SB], BF16, tag="onesrow")
        nc.vector.memset(ones_row[:, :], 0.0)
        nc.vector.memset(ones_row[0:1, :], 1.0)

        whh0 = wpool.tile([128, KH, G], BF16, tag="whh", name="whh0")
        nc.sync.dma_start(out=whh0[:, :, :], in_=w["whh0"][:, :, :])
        hdst = rpool.tile([128, KH, SB], BF16, tag="hdst", name="hdst")
        hdst2 = rpool.tile([128, KH, SB], BF16, tag="hdst2", name="hdst2")

        filler = _Filler()

        def scan_layer(whh_sb, xw, hd, hd2, rev=False):
            """Single-direction 256-step scan with gap-filler items. With
            rev=True the scan walks its local columns backwards (static
            indexing), i.e. computes the opposite LSTM direction."""
            hbuf = rpool.tile([128, 2, KH, BL], BF16, tag="hbuf", name="hbuf")
            nc.vector.memset(hbuf[:, 0, :, :], 0.0)
            cbuf = rpool.tile([128, KH, BL], F32, tag="cbuf", name="cbuf")
            nc.vector.memset(cbuf[:, :, :], 0.0)

            for j in range(S):
                filler.step(j)
                cur, nxt = j % 2, (j + 1) % 2
                cols = (S - 1 - j) * BL if rev else j * BL
                gorder = (3, 0, 1, 2)          # g, i, f, o
                pg, gs = {}, {}
                for gg in gorder:
                    pg[gg] = psg.tile([128, KH, BL], F32, tag="gates",
                                      name=f"pg{gg}")
                    for mm in range(KH):
                        m = 4 * gg + mm
                        for k in range(KH):
                            nc.tensor.matmul(
                                pg[gg][:, mm, :],
                                whh_sb[:, k, 128 * m:128 * (m + 1)],
                                hbuf[:, cur, k, :],
                                start=(k == 0), stop=(k == KH - 1))
                    gs[gg] = scpool.tile([128, KH, BL], F32, tag=f"gs{gg}",
                                         name=f"gs{gg}")
                    nc.vector.tensor_tensor(
                        gs[gg][:, :, :], pg[gg][:, :, :],
                        xw[:, 4 * gg:4 * gg + 4, ds(cols, BL)], OP.add)
                    if gg == 3:
                        tg = scpool.tile([128, KH, BL], F32, tag="tg",
                                         name="tg")
                        nc.scalar.activation(tg[:, :, :], gs[3][:, :, :],
                                             AF.Tanh)
                    elif gg == 0:
                        si = scpool.tile([128, KH, BL], F32, tag="si")
                        nc.scalar.activation(si[:, :, :], gs[0][:, :, :],
                                             AF.Sigmoid)
                        t1 = scpool.tile([128, KH, BL], F32, tag="t1",
                                         name="t1")
                        nc.vector.tensor_tensor(t1[:, :, :], si[:, :, :],
                                                tg[:, :, :], OP.mult)
                    elif gg == 1:
                        sf = scpool.tile([128, KH, BL], F32, tag="sf")
                        nc.scalar.activation(sf[:, :, :], gs[1][:, :, :],
                                             AF.Sigmoid)
                        t2 = scpool.tile([128, KH, BL], F32, tag="t2")
                        nc.vector.tensor_tensor(t2[:, :, :], sf[:, :, :],
                                                cbuf[:, :, :], OP.mult)
                        nc.vector.tensor_tensor(cbuf[:, :, :], t1[:, :, :],
                                                t2[:, :, :], OP.add)
                        tcc = scpool.tile([128, KH, BL], F32, tag="tcc",
                                          name="tcc")
                        nc.scalar.activation(tcc[:, :, :], cbuf[:, :, :],
                                             AF.Tanh)
                    else:
                        so = scpool.tile([128, KH, BL], F32, tag="so",
                                         name="so")
                        nc.scalar.activation(so[:, :, :], gs[2][:, :, :],
                                             AF.Sigmoid)
                        nc.vector.tensor_tensor(hbuf[:, nxt, :, :],
                                                so[:, :, :], tcc[:, :, :],
                                                OP.mult)
                nc.vector.tensor_tensor(hd[:, :, ds(cols, BL)],
                                        so[:, :, :], tcc[:, :, :], OP.mult)
                if hd2 is not None:
                    nc.vector.tensor_tensor(
                        hd2[:, :, ds((S - 1 - j) * BL, BL)],
                        so[:, :, :], tcc[:, :, :], OP.mult)

        # ---- layer 0: x.T features + projection, chunk-pipelined ---------
        xw0 = xwpool.tile([128, MT, SB], BF16, tag="xw", name="xw0")
        stx = st.enter_context(ExitStack())
        xtpool = stx.enter_context(tc.tile_pool(name="xtp", bufs=1))
        hlpool = stx.enter_context(tc.tile_pool(name="hl", bufs=5))
        sumpool = stx.enter_context(tc.tile_pool(name="sum", bufs=4))
        frpool = stx.enter_context(tc.tile_pool(name="frp", bufs=1))

        xt = xtpool.tile([128, K0, SB], BF16, tag="xt")
        hs_sbe = hs.rearrange("l b s e -> l s b e")
        rt_sum = {}

        def rowtile_dma(r):
            hl = []
            for layer in range(4):
                t = hlpool.tile([128, E], BF16, tag="hl")
                nc.sync.dma_start(
                    out=t[:, :],
                    in_=hs_sbe[layer, TPR * r:TPR * (r + 1), :, :])
                hl.append(t)
            s01 = sumpool.tile([128, E], F32, tag="sum")
            nc.vector.tensor_tensor(s01[:, :], hl[0][:, :], hl[1][:, :],
                                    OP.add)
            s23 = sumpool.tile([128, E], F32, tag="sum")
            nc.vector.tensor_tensor(s23[:, :], hl[2][:, :], hl[3][:, :],
                                    OP.add)
            ssum = sumpool.tile([128, E], F32, tag="sum")
            nc.vector.tensor_tensor(ssum[:, :], s01[:, :], s23[:, :], OP.add)
            rt_sum[r] = ssum

        def rowtile_tp(r):
            ssum = rt_sum.pop(r)
            for c in range(6):
                pt = pspool.tile([128, 128], F32, tag="tp")
                nc.tensor.transpose(pt[:, :], ssum[:, 128 * c:128 * (c + 1)],
                                    ident[:, :])
                nc.vector.tensor_scalar_mul(
                    xt[:, c, 128 * r:128 * (r + 1)], pt[:, :], 0.25)

        def proj_item(xw, ch, segs, off=0, ln=512):
            """One m-tile, one column sub-range of a 512-col chunk: weight
            DMAs + one accumulation chain + PSUM->SBUF copy. segs: list of
            (wih_dram, nk, rhs_of_k, m). Returns a thunk."""
            def emit():
                wms = []
                for (wih_dram, nk, _, m) in segs:
                    wm = wpool.tile([128, nk, 128], BF16, tag=f"wihm{nk}")
                    nc.sync.dma_start(out=wm[:, :, :], in_=wih_dram[m])
                    wms.append(wm)
                pp = psproj.tile([128, 512], F32, tag="proj")
                nks = sum(s[1] for s in segs)
                kk = 0
                for wm, (_, nk, rhs_of_k, m) in zip(wms, segs):
                    for k in range(nk):
                        nc.tensor.matmul(pp[:, 0:ln],
                                         wm[:, k, :],
                                         rhs_of_k(k, ch)[:, off:off + ln],
                                         start=(kk == 0), stop=(kk == nks - 1))
                        kk += 1
                m0 = segs[0][3]
                base = 512 * ch + off
                nc.vector.tensor_copy(xw[:, m0, base:base + ln],
                                      pp[:, 0:ln])
            return emit

        def xt_rhs(k, ch):
            return xt[:, k, 512 * ch:512 * (ch + 1)]

        # feature rows (delta@p0, mask@p32, ones@p64) in xt[:, 6, :]
        nc.vector.memset(xt[:, 6, :], 0.0)
        nc.vector.memset(xt[64:65, 6, :], 1.0)
        frow = frpool.tile([1, SB], F32, tag="frow", name="frow_r")
        nc.sync.dma_start(out=frow[:, :], in_=rmask[:, :])
        nc.vector.tensor_copy(xt[32:33, 6, :], frow[:, :])
        frow2 = frpool.tile([1, SB], F32, tag="frow", name="frow_d")
        nc.sync.dma_start(out=frow2[:, :], in_=drow[:, :])
        nc.vector.tensor_copy(xt[0:1, 6, :], frow2[:, :])

        # Up front: only row-tile 0 + the first 128 projection columns (16
        # scan steps of lead); the rest of chunk 0 fills the earliest scan
        # gaps. Chunks 1-3 fill the remaining stall gaps.
        for r in range(RPC):
            rowtile_dma(r)
        rowtile_tp(0)
        for m in range(MT):
            proj_item(xw0, 0, [(w["wih0"], K0, xt_rhs, m)], 0, 128)()
        for r in range(1, RPC):
            filler.add(0, r, (lambda rr: lambda: rowtile_tp(rr))(r))
        for m in range(MT):
            filler.add(0, 4 + (10 * m) // MT,
                       proj_item(xw0, 0, [(w["wih0"], K0, xt_rhs, m)],
                                 128, 384))
        for c in range(1, NCH):
            for r in range(RPC * c, RPC * (c + 1)):
                filler.add(SPC * (c - 1), SPC * c - 8,
                           (lambda rr: lambda: rowtile_dma(rr))(r))
                filler.add(SPC * (c - 1), SPC * c - 4,
                           (lambda rr: lambda: rowtile_tp(rr))(r))
            for m in range(MT):
                filler.add(SPC * (c - 1), SPC * c,
                           proj_item(xw0, c, [(w["wih0"], K0, xt_rhs, m)]))

        # The upper half of hdst2 (cols for t >= S/2) is written by scan
        # steps 0..S/2-1, so its AllReduce can run during the scan's second
        # half (both pair cores reach step S/2 at about the same time).
        def exchange_hi():
            nc.sync.dma_start(out=hx["in_hi"].ap(),
                              in_=hdst2[:, :, SB // 2:])
            nc.gpsimd.collective_compute(
                "AllReduce", OP.add, replica_groups=RG,
                ins=[hx["in_hi"].ap().opt()], outs=[hx["out_hi"].ap().opt()])

        filler.add(S // 2, S // 2 + 8, exchange_hi)

        scan_layer(whh0, xw0, hdst, hdst2)
        filler.drain()
        stx.close()
        latepool = st.enter_context(tc.tile_pool(name="late", bufs=1))

        # ---- exchange lower half; hrecv = sum - own ---------------------
        nc.sync.dma_start(out=hx["in_lo"].ap(), in_=hdst2[:, :, :SB // 2])
        nc.gpsimd.collective_compute(
            "AllReduce", OP.add, replica_groups=RG,
            ins=[hx["in_lo"].ap().opt()], outs=[hx["out_lo"].ap().opt()])

        # L1 own-half chunk-0 projection overlaps the collective on the PE
        whh1 = wpool.tile([128, KH, G], BF16, tag="whh", name="whh1")
        nc.sync.dma_start(out=whh1[:, :, :], in_=w["whh1"][:, :, :])
        wo = wpool.tile([128, K1O, C], BF16, tag="wout")
        nc.sync.dma_start(out=wo[:, :, :], in_=w["wout"][:, :, :])

        def own_rhs(k, ch):
            if k < KH:
                return hdst[:, k, 512 * ch:512 * (ch + 1)]
            return ones_row[:, 512 * ch:512 * (ch + 1)]

        def rcv_rhs(k, ch):
            return hrecv[:, k, 512 * ch:512 * (ch + 1)]

        hrecv = latepool.tile([128, KH, SB], BF16, tag="hrecv", name="hrecv")
        nc.sync.dma_start(out=hrecv[:, :, SB // 2:], in_=hx["out_hi"].ap())
        nc.vector.tensor_tensor(hrecv[:, :, SB // 2:],
                                hrecv[:, :, SB // 2:],
                                hdst2[:, :, SB // 2:], OP.subtract)

        # L1 scan (reversed) consumes chunk NCH-1 first; its own+recv data
        # (high-t) is ready now -- cc#1 completed during the L0 scan.
        xw1 = xwpool.tile([128, MT, SB], BF16, tag="xw", name="xw1")
        for m in range(MT):
            proj_item(xw1, NCH - 1, [(w["wih1o"], K1O, own_rhs, m)])()

        def rcv_acc_item(m, ch):
            def emit():
                wm = wpool.tile([128, K1R, 128], BF16, tag=f"wihm{K1R}")
                nc.sync.dma_start(out=wm[:, :, :], in_=w["wih1r"][m])
                pp = psproj.tile([128, 512], F32, tag="proj")
                for k in range(K1R):
                    nc.tensor.matmul(pp[:, :], wm[:, k, :], rcv_rhs(k, ch),
                                     start=(k == 0), stop=(k == K1R - 1))
                nc.vector.tensor_tensor(
                    xw1[:, m, 512 * ch:512 * (ch + 1)], pp[:, :],
                    xw1[:, m, 512 * ch:512 * (ch + 1)], OP.add)
            return emit

        for m in range(MT):
            rcv_acc_item(m, NCH - 1)()

        def recv_lo():
            nc.sync.dma_start(out=hrecv[:, :, :SB // 2],
                              in_=hx["out_lo"].ap())
            nc.vector.tensor_tensor(hrecv[:, :, :SB // 2],
                                    hrecv[:, :, :SB // 2],
                                    hdst2[:, :, :SB // 2], OP.subtract)

        filler.add(24, 56, recv_lo)

        h1 = latepool.tile([128, KH, SB], BF16, tag="h1", name="h1")
        out_sbc = out.rearrange("b s c -> s b c")

        def outproj_item(ch):
            def emit():
                po = psproj.tile([C, 512], F32, tag="proj")
                for k in range(K1O):
                    if k < KH:
                        rhs = h1[:, k, 512 * ch:512 * (ch + 1)]
                    else:
                        rhs = ones_row[:, 512 * ch:512 * (ch + 1)]
                    nc.tensor.matmul(po[:, :], wo[:, k, :], rhs,
                                     start=(k == 0), stop=(k == K1O - 1))
                ost = scpool.tile([C, 512], F32, tag="ost")
                nc.vector.tensor_copy(ost[:, :], po[:, :])
                for cb in range(4):
                    pt = pspool.tile([128, C], F32, tag="tp")
                    nc.tensor.transpose(pt[:, :],
                                        ost[:, 128 * cb:128 * (cb + 1)],
                                        ident[0:C, 0:C])
                    onat = scpool.tile([128, C], F32, tag="onat")
                    nc.vector.tensor_copy(onat[:, :], pt[:, :])
                    gb = 4 * ch + cb
                    nc.sync.dma_start(
                        out=out_sbc[TPR * gb:TPR * (gb + 1), :, :],
                        in_=onat[:, :])
            return emit

        # L1 fillers: remaining xw1 chunks (own+recv 9-chains) trail one
        # chunk ahead of the scan; out-projection chunks trail completion.
        for idx, c in enumerate(range(NCH - 2, -1, -1)):
            for m in range(MT):
                filler.add(SPC * idx, SPC * (idx + 1),
                           proj_item(xw1, c,
                                     [(w["wih1o"], K1O, own_rhs, m),
                                      (w["wih1r"], K1R, rcv_rhs, m)]))
        for idx, c in enumerate(range(NCH - 1, 0, -1)):
            filler.add(SPC * (idx + 1), 10 ** 9, outproj_item(c))
        filler.add(10 ** 9, 10 ** 9, outproj_item(0))

        scan_layer(whh1, xw1, h1, None, rev=True)
        filler.drain()


def _get_nc():
    if "nc" not in _cache:
        _cache["nc"] = build_nc()
    return _cache["nc"]


def make_in_maps(inputs):
    wmaps = {d: _prep_weights(inputs, d) for d in ("f", "b")}
    return [_prep_core_inputs(inputs, wmaps, c) for c in range(NCORES)]


def kernel(**inputs):
    from concourse.bass_utils import run_bass_kernel_spmd

    in_maps = make_in_maps(inputs)
    nc = _get_nc()
    res = run_bass_kernel_spmd(nc, in_maps, core_ids=list(range(NCORES)))
    parts = [r["out"] for r in res.results]
    full = np.empty((B, S, C), np.float32)
    for p in range(NPAIR):
        full[BL * p:BL * (p + 1)] = parts[2 * p] + parts[2 * p + 1][:, ::-1]
    return full
